# revision 19
# baseline (speedup 1.0000x reference)
"""CGCNN / GENConv GNN message-passing kernel for 8 Trainium2 NeuronCores.

Strategy (dst-sharded edge parallel, load-balanced windows):
  - Host assigns nodes to 240 windows of 128 slots (30720 padded table) with
    a greedy balance on in-degree so every window receives ~2000 edges
    (T = 16 tiles of 128 edges).  Core k owns windows [k*30, (k+1)*30).
    Segment softmax/sums never cross cores.
  - Each layer: every core gathers h[src] for its edges from a replicated
    node table in its local DRAM (dma_gather, 256B rows, single_packet=False
    so the 16 SDMA engines pipeline the random HBM reads), computes
    msg = relu(h_src + ea) + eps, e = exp(t*msg), me = msg*e, and
    segment-reduces [sum e | sum me] per 128-node window with a one-hot
    matmul accumulated in PSUM.  agg = (sum me) / (sum e + eps) (algebraic
    equivalent of the reference segment softmax).
  - Node MLP / LayerNorm runs data-parallel on the core's own node shard
    (bn_stats/bn_aggr + reciprocal_approx_fast LN).
  - The updated table is AllGather'ed in two chunks per layer so the
    collective overlaps the tail windows' compute.

kernel(**inputs) takes the FULL inputs and returns the FULL [30000, 10]
output; sharding + compilation happen inside (compiled program is cached).
"""

import os
import sys

sys.path.insert(0, "/opt/trn_rl_repo")

import numpy as np

import concourse.bacc as bacc
import concourse.bass as bass
import concourse.mybir as mybir
import concourse.tile as tile
from concourse.bass_utils import run_bass_kernel_spmd
from concourse.library_config import mlp as mlp_lib

F32 = mybir.dt.float32
F16 = mybir.dt.float16
I32 = mybir.dt.int32
AF = mybir.ActivationFunctionType
ALU = mybir.AluOpType

MSG_EPS = 1e-7
SM_EPS = 1e-16
LN_EPS = 1e-5

# problem dims (hardcoded per harness contract)
N_NODES = 30000
N_EDGES = 480000
F_IN = 64
F_EDGE = 16
HID = 128
N_LAYERS = 3
N_CLASSES = 10
CORES = 8
WIN = 128
W = 30                     # windows per core
NSLOT = CORES * W * WIN    # padded node table (30720)
NPC = W * WIN              # padded nodes per core (3840)

ME_ENGINE = os.environ.get("K_ME_ENGINE", "gpsimd")  # gpsimd | vector
SINGLE_PACKET = os.environ.get("K_SINGLE_PACKET", "0") == "1"
AG_CHUNKS = int(os.environ.get("K_AG_CHUNKS", "1"))


# --------------------------------------------------------------------------
# host-side sharding / packing
# --------------------------------------------------------------------------

def _balance_nodes(dst, n_nodes):
    """Assign nodes to CORES*W windows of WIN slots, balancing edge counts.

    Returns newid[orig_node] -> padded slot id in [0, NSLOT).
    """
    import heapq

    nwin = CORES * W
    indeg = np.bincount(dst, minlength=n_nodes).astype(np.int64)
    order = np.argsort(-indeg, kind="stable")
    heap = [(0, 0, w) for w in range(nwin)]  # (load, used, win)
    heapq.heapify(heap)
    newid = np.empty(n_nodes, np.int64)
    # process nodes in blocks of nwin for speed: pop all, assign, push back
    i = 0
    while i < n_nodes:
        k = min(nwin, len(heap), n_nodes - i)
        block = order[i:i + k]
        entries = [heapq.heappop(heap) for _ in range(k)]
        kept = []
        for j, node in enumerate(block):
            load, used, w = entries[j]
            newid[node] = w * WIN + used
            load += indeg[node]
            used += 1
            if used < WIN:
                kept.append((load, used, w))
        for e in kept:
            heapq.heappush(heap, e)
        i += k
    return newid


def _prep_edges(edge_index, edge_attr, n_nodes):
    """Permute + sort edges by dst window, pack per (core, window, tile)."""
    src = edge_index[0].astype(np.int64)
    dst = edge_index[1].astype(np.int64)
    newid = _balance_nodes(dst, n_nodes)
    srcp = newid[src]
    dstp = newid[dst]

    order = np.argsort(dstp, kind="stable")
    srcp = srcp[order]
    dstp = dstp[order]
    attr = edge_attr[order]

    gwin = dstp // WIN  # global window of each edge, non-decreasing
    counts = np.bincount(gwin, minlength=CORES * W)
    T = int(np.ceil(counts.max() / 128.0))
    T = max(T, 1)
    epw = T * 128
    starts = np.zeros(CORES * W + 1, np.int64)
    np.cumsum(counts, out=starts[1:])

    fe = attr.shape[1]
    src_pad = np.zeros((CORES, W, epw), np.int64)
    dstloc_pad = np.full((CORES, W, epw), -1.0, np.float16)
    attr_pad = np.zeros((CORES, W, epw, fe), np.float32)
    for c in range(CORES):
        for w in range(W):
            g = c * W + w
            s, e = starts[g], starts[g + 1]
            n = e - s
            src_pad[c, w, :n] = srcp[s:e]
            dstloc_pad[c, w, :n] = (dstp[s:e] - g * WIN).astype(np.float16)
            attr_pad[c, w, :n, :] = attr[s:e]

    # gather index layout: [128, W*T*8] int16, idx i of window w at
    # partition i%16 (replicated x8), column w*T*8 + i//16
    sp = src_pad.reshape(CORES, W, T * 8, 16)
    gidx16 = np.transpose(sp, (0, 3, 1, 2)).reshape(CORES, 16, W * T * 8)
    gidx = np.tile(gidx16, (1, 8, 1)).astype(np.int16)

    # dstloc: [128, W*T] f16, tile j=w*T+g column, partition = edge offset
    dl = dstloc_pad.reshape(CORES, W, T, 128)
    dstloc = np.ascontiguousarray(
        np.transpose(dl, (0, 3, 1, 2)).reshape(CORES, 128, W * T)
    )

    # attrT: [17, EPAD]: rows 0..15 features (transposed), row 16 = ones
    ap = attr_pad.reshape(CORES, W * epw, fe)
    attrT = np.empty((CORES, fe + 1, W * epw), np.float16)
    attrT[:, :fe, :] = np.transpose(ap, (0, 2, 1))
    attrT[:, fe, :] = 1.0
    return T, newid, gidx, dstloc, np.ascontiguousarray(attrT)


def _prep_inputs(inputs):
    """Build the 8 per-core input maps (and shared weight arrays)."""
    x = np.asarray(inputs["x"], np.float32)
    edge_attr = np.asarray(inputs["edge_attr"], np.float32)
    edge_index = np.asarray(inputs["edge_index"])
    n_nodes, fin = x.shape

    T, newid, gidx, dstloc, attrT = _prep_edges(edge_index, edge_attr, n_nodes)

    L = int(np.asarray(inputs["t"]).shape[0])
    hid = np.asarray(inputs["node_enc_w"]).shape[1]
    H2 = 2 * hid

    # permuted node features: xp[newid[i]] = x[i]; empty slots zero
    xp = np.zeros((NSLOT, fin), np.float32)
    xp[newid] = x
    xs = xp.reshape(CORES, NPC, fin)
    xT = np.empty((CORES, fin + 1, NPC), np.float32)
    xT[:, :fin, :] = np.transpose(xs, (0, 2, 1))
    xT[:, fin, :] = 1.0

    wnode = np.concatenate(
        [np.asarray(inputs["node_enc_w"], np.float32),
         np.asarray(inputs["node_enc_b"], np.float32)[None, :]], 0
    )
    wenc = np.concatenate(
        [np.asarray(inputs["edge_enc_w"], np.float32),
         np.asarray(inputs["edge_enc_b"], np.float32)[None, :]], 0
    ).astype(np.float16)
    w1 = np.ascontiguousarray(inputs["mlp1_w"], np.float16)      # [L,H,2H]
    w2 = np.ascontiguousarray(inputs["mlp2_w"], np.float16)      # [L,2H,H]

    def bc(a, n):
        return np.ascontiguousarray(
            np.broadcast_to(np.asarray(a, np.float16)[:, None, :], (L, 128, n)))

    b1bc = bc(inputs["mlp1_b"], H2)
    b2bc = bc(inputs["mlp2_b"], hid)
    g1bc = bc(inputs["mlp_ln_g"], H2)
    bb1bc = bc(inputs["mlp_ln_b"], H2)
    ngbc = bc(inputs["norm_g"], hid)
    nbbc = bc(inputs["norm_b"], hid)
    tcol = np.ascontiguousarray(
        np.broadcast_to(np.asarray(inputs["t"], np.float32)[None, :], (128, L)))
    linw = np.ascontiguousarray(inputs["lin_w"], np.float16)
    linbbc = np.ascontiguousarray(
        np.broadcast_to(np.asarray(inputs["lin_b"], np.float16)[None, :],
                        (128, N_CLASSES)))
    iota = np.ascontiguousarray(
        np.broadcast_to(np.arange(128, dtype=np.float16)[None, :], (128, 128)))
    ident = np.eye(128, dtype=np.float32)
    identh = np.eye(128, dtype=np.float16)

    shared = dict(wnode=wnode, wench=wenc, w1h=w1, w2h=w2, b1bc=b1bc,
                  b2bc=b2bc, g1bc=g1bc, bb1bc=bb1bc, ngbc=ngbc, nbbc=nbbc,
                  tcol=tcol, linwh=linw, linbbc=linbbc, iota=iota,
                  ident=ident, identh=identh)
    in_maps = []
    for c in range(CORES):
        m = dict(shared)
        m["xT"] = np.ascontiguousarray(xT[c])
        m["attrT"] = attrT[c]
        m["gidx"] = np.ascontiguousarray(gidx[c])
        m["dstloc"] = dstloc[c]
        in_maps.append(m)
    return T, newid, in_maps


# --------------------------------------------------------------------------
# device program
# --------------------------------------------------------------------------

def build_program(T, fin=F_IN, fe=F_EDGE, hid=HID, L=N_LAYERS, ncls=N_CLASSES):
    EPAD = W * T * 128
    H2 = 2 * hid
    npc = NPC
    nslot = NSLOT
    nc = bacc.Bacc("TRN2", target_bir_lowering=False, debug=False,
                   num_devices=CORES, dynamic_dma_scratch_size=131072,
                   num_swdge_queues=4)

    d = nc.dram_tensor
    xT_d = d("xT", [fin + 1, npc], F32, kind="ExternalInput")
    attrT_d = d("attrT", [fe + 1, EPAD], F16, kind="ExternalInput")
    gidx_d = d("gidx", [128, W * T * 8], mybir.dt.int16, kind="ExternalInput")
    dstloc_d = d("dstloc", [128, W * T], F16, kind="ExternalInput")
    wnode_d = d("wnode", [fin + 1, hid], F32, kind="ExternalInput")
    wenc_d = d("wench", [fe + 1, hid], F16, kind="ExternalInput")
    w1_d = d("w1h", [L, hid, H2], F16, kind="ExternalInput")
    w2_d = d("w2h", [L, H2, hid], F16, kind="ExternalInput")
    b1bc_d = d("b1bc", [L, 128, H2], F16, kind="ExternalInput")
    b2bc_d = d("b2bc", [L, 128, hid], F16, kind="ExternalInput")
    g1bc_d = d("g1bc", [L, 128, H2], F16, kind="ExternalInput")
    bb1bc_d = d("bb1bc", [L, 128, H2], F16, kind="ExternalInput")
    ngbc_d = d("ngbc", [L, 128, hid], F16, kind="ExternalInput")
    nbbc_d = d("nbbc", [L, 128, hid], F16, kind="ExternalInput")
    tcol_d = d("tcol", [128, L], F32, kind="ExternalInput")
    linw_d = d("linwh", [hid, ncls], F16, kind="ExternalInput")
    linbbc_d = d("linbbc", [128, ncls], F16, kind="ExternalInput")
    iota_d = d("iota", [128, 128], F16, kind="ExternalInput")
    ident_d = d("ident", [128, 128], F32, kind="ExternalInput")
    identh_d = d("identh", [128, 128], F16, kind="ExternalInput")
    out_d = d("out", [npc, ncls], F32, kind="ExternalOutput")

    rg = [list(range(CORES))]
    NQ = (T + 3) // 4          # quads of <=4 tiles per window
    NG = (T + 7) // 8          # gather calls of <=8 tiles per window
    ag_chunk = (W + AG_CHUNKS - 1) // AG_CHUNKS

    with tile.TileContext(nc) as tc:
        nc.gpsimd.load_library(mlp_lib)
        with (
            tc.tile_pool(name="const", bufs=1) as cp,
            tc.tile_pool(name="sbw", bufs=2) as sbw,       # window tiles
            tc.tile_pool(name="sbm", bufs=2) as sbm,       # MLP scratch
            tc.tile_pool(name="psq", bufs=2, space="PSUM") as psq,
            tc.tile_pool(name="psa", bufs=2, space="PSUM") as psa,
            tc.tile_pool(name="psu", bufs=2, space="PSUM") as psu,
            tc.tile_pool(name="pst", bufs=1, space="PSUM") as pst,
            tc.tile_pool(name="pso", bufs=1, space="PSUM") as pso,
            tc.tile_pool(name="dram", bufs=2, space="DRAM") as dp,
            tc.tile_pool(name="dram1", bufs=1, space="DRAM") as dp1,
        ):
            # ---------------- constants / weights to SBUF ----------------
            def load(name, dram_ap, shape, rearr=None, dt_=F32, **kw):
                t = cp.tile(shape, dt_, tag=name)
                src = dram_ap if rearr is None else dram_ap.rearrange(rearr, **kw)
                nc.sync.dma_start(t[:], src)
                return t

            wnode_s = load("wnode", wnode_d[:, :], [fin + 1, hid])
            wenc_s = load("wenc", wenc_d[:, :], [fe + 1, hid], dt_=F16)
            w1_s = load("w1", w1_d[:, :, :], [hid, L, H2], "l k n -> k l n",
                        dt_=F16)
            w2_s = load("w2", w2_d[:, :, :], [128, L, 2, hid],
                        "l (h k) n -> k l h n", h=2, dt_=F16)
            b1_s = load("b1", b1bc_d[:, :, :], [128, L, H2], "l p n -> p l n",
                        dt_=F16)
            b2_s = load("b2", b2bc_d[:, :, :], [128, L, hid], "l p n -> p l n",
                        dt_=F16)
            g1_s = load("g1", g1bc_d[:, :, :], [128, L, H2], "l p n -> p l n",
                        dt_=F16)
            bb1_s = load("bb1", bb1bc_d[:, :, :], [128, L, H2],
                         "l p n -> p l n", dt_=F16)
            ng_s = load("ng", ngbc_d[:, :, :], [128, L, hid], "l p n -> p l n",
                        dt_=F16)
            nb_s = load("nb", nbbc_d[:, :, :], [128, L, hid], "l p n -> p l n",
                        dt_=F16)
            tcol_s = load("tcol", tcol_d[:, :], [128, L])
            linw_s = load("linw", linw_d[:, :], [hid, ncls], dt_=F16)
            linb_s = load("linb", linbbc_d[:, :], [128, ncls], dt_=F16)
            iota_s = load("iota", iota_d[:, :], [128, 128], dt_=F16)
            ident_s = load("ident", ident_d[:, :], [128, 128])
            identh_s = load("identh", identh_d[:, :], [128, 128], dt_=F16)
            dstloc_s = load("dstloc", dstloc_d[:, :], [128, W * T], dt_=F16)
            gidx_s = cp.tile([128, W * T * 8], mybir.dt.int16, tag="gidx")
            nc.sync.dma_start(gidx_s[:], gidx_d[:, :])

            ln_col = cp.tile([128, 1], F32, tag="lnc")
            nc.vector.memset(ln_col[:], LN_EPS)

            h_state = dp1.tile([npc, hid], F32)

            # ---------------- helpers ----------------
            def layer_norm(dst_ap, src_ap, g_ap, b_ap, tag, relu_eng):
                """dst <- relu(LN(src) * g + b).  src must be SBUF/PSUM f32."""
                st6 = sbm.tile([128, 6], F32, tag="lnst" + tag)
                nc.vector.bn_stats(st6[:], src_ap)
                ag2 = sbm.tile([128, 2], F32, tag="lnag" + tag)
                nc.vector.bn_aggr(ag2[:], st6[:])
                sd = sbm.tile([128, 1], F32, tag="lnsd" + tag)
                nc.scalar.activation(sd[:], ag2[:, 1:2], AF.Sqrt,
                                     bias=ln_col[:])
                rstd = sbm.tile([128, 1], F32, tag="lnrs" + tag)
                nc.vector.reciprocal_approx_fast(rstd[:], sd[:])
                nfeat = src_ap.shape[-1]
                y = sbm.tile([128, nfeat], F32, tag="lny" + tag)
                nc.vector.tensor_scalar(y[:], src_ap, ag2[:, 0:1], rstd[:],
                                        op0=ALU.subtract, op1=ALU.mult)
                nc.vector.tensor_tensor(y[:], y[:], g_ap, op=ALU.mult)
                nc.vector.tensor_tensor(y[:], y[:], b_ap, op=ALU.add)
                relu_eng(dst_ap, y[:])

            def srelu(dst_ap, src_ap):
                nc.scalar.activation(dst_ap, src_ap, AF.Relu)

            def vrelu(dst_ap, src_ap):
                nc.vector.tensor_scalar_max(dst_ap, src_ap, 0.0)

            def transpose128(src_ap, tag, copy_eng="scalar"):
                """PE transpose [128,128] f32 -> SBUF f16."""
                pt = pst.tile([128, 128], F32, tag="pt")
                nc.tensor.transpose(pt[:], src_ap, ident_s[:])
                st = sbm.tile([128, 128], F16, tag=tag)
                if copy_eng == "scalar":
                    nc.scalar.activation(st[:], pt[:], AF.Copy)
                else:
                    nc.vector.tensor_copy(st[:], pt[:])
                return st

            # ---------------- encode phase: h0 = x @ wnode ----------------
            def new_ag(nm):
                return dp.tile([npc, hid], F16, tag="ag", name=nm)

            def new_table(nm):
                return dp.tile([nslot, hid], F16, tag="table",
                               addr_space="Shared", name=nm)

            def emit_ag(agp, table, chunk):
                base = chunk * ag_chunk * 128
                rows = min(ag_chunk * 128, npc - base)
                tview = table[:, :].rearrange("(c n) h -> c n h", c=CORES)
                nc.gpsimd.collective_compute(
                    "AllGather", ALU.bypass,
                    ins=[agp[base:base + rows, :].opt()],
                    outs=[tview[:, base:base + rows, :].opt()],
                    replica_groups=rg)

            ag = new_ag("age")
            table = new_table("tbl0")
            for w in range(W):
                base = w * 128
                xts = sbm.tile([fin + 1, 128], F32, tag="xts")
                nc.sync.dma_start(xts[:], xT_d[:, base:base + 128])
                ph = pso.tile([128, hid], F32, tag="po")
                nc.tensor.matmul(ph[:], xts[:], wnode_s[:],
                                 start=True, stop=True)
                h0 = sbm.tile([128, hid], F16, tag="h0")
                nc.vector.tensor_copy(h0[:], ph[:])
                nc.sync.dma_start(ag[base:base + 128, :], h0[:])
                if (w + 1) % ag_chunk == 0 or w == W - 1:
                    emit_ag(ag, table, w // ag_chunk)

            # ---------------- conv layers ----------------
            for li in range(L):
                ag_next = new_ag(f"agn{li}") if li < L - 1 else None
                next_table = new_table(f"tbl{li + 1}") if li < L - 1 else None
                for w in range(W):
                    base = w * 128
                    jw = w * T

                    # gather h[src] for this window: [128, T, 128]
                    hsrc = sbw.tile([128, T, 128], F16, tag="hsrc", bufs=3)
                    for qi in range(NG):
                        c0 = qi * 8
                        ct = min(8, T - c0)
                        nc.gpsimd.dma_gather(
                            hsrc[:, c0:c0 + ct, :], table[:, :],
                            gidx_s[:, (jw + c0) * 8:(jw + c0 + ct) * 8],
                            ct * 128, ct * 128, hid,
                            queue_num=(w * NG + qi) % 4,
                            single_packet=SINGLE_PACKET)
                    attrs = sbw.tile([fe + 1, T, 128], F16, tag="attrs")
                    nc.sync.dma_start(
                        attrs[:], attrT_d[:, jw * 128:(jw + T) * 128])

                    msg = sbw.tile([128, T, 128], F16, tag="msg")
                    em = sbw.tile([128, 2, T, 128], F16, tag="em", bufs=2)
                    # quads: ea matmul + gathered-h add (identity matmul)
                    for q in range(NQ):
                        q0 = q * 4
                        qs = min(4, T - q0)
                        pq = psq.tile([128, 4, 128], F32, tag="pq")
                        for j in range(qs):
                            nc.tensor.matmul(
                                pq[:, j, :], attrs[:, q0 + j, :], wenc_s[:],
                                start=(j == 0), stop=False,
                                skip_group_check=True)
                        nc.tensor.matmul(
                            pq[:, :qs, :], identh_s[:],
                            hsrc[:, q0:q0 + qs, :],
                            start=False, stop=True, skip_group_check=True)
                        # msg = relu(ea + h_src) + eps  (one fused DVE op)
                        nc.vector.tensor_scalar(
                            msg[:, q0:q0 + qs, :], pq[:, :qs, :], 0.0,
                            MSG_EPS, op0=ALU.max, op1=ALU.add)
                    # e = exp(t * msg) ; me = msg * e
                    nc.scalar.activation(
                        em[:, 0, :, :], msg[:], AF.Exp,
                        scale=tcol_s[:, li:li + 1])
                    me_eng = nc.gpsimd if ME_ENGINE == "gpsimd" else nc.vector
                    me_eng.tensor_tensor(
                        em[:, 1, :, :], msg[:], em[:, 0, :, :], op=ALU.mult)
                    # one-hot S for the whole window (all-f16)
                    S = sbw.tile([128, T, 128], F16, tag="S")
                    iota_b = iota_s[:].rearrange(
                        "p (o f) -> p o f", o=1).broadcast_to([128, T, 128])
                    dl_b = dstloc_s[:, jw:jw + T].rearrange(
                        "p (t o) -> p t o", o=1).broadcast_to([128, T, 128])
                    nc.vector.tensor_tensor(S[:], iota_b, dl_b,
                                            op=ALU.is_equal)
                    # segment accumulate [sum e | sum me] -> [128, 256] psum
                    acc = psa.tile([128, 2, hid], F32, tag="acc")
                    for g in range(T):
                        nc.tensor.matmul(
                            acc[:, :, :], S[:, g, :], em[:, :, g, :],
                            start=(g == 0), stop=(g == T - 1))

                    # agg = (sum me) / (sum e + eps)
                    sep = sbm.tile([128, hid], F32, tag="sep")
                    nc.vector.tensor_scalar_add(sep[:], acc[:, 0, :], SM_EPS)
                    rcse = sbm.tile([128, hid], F32, tag="rcse")
                    nc.vector.reciprocal_approx_fast(rcse[:], sep[:])
                    zmul = sbm.tile([128, hid], F32, tag="zmul")
                    nc.vector.tensor_tensor(zmul[:], acc[:, 1, :], rcse[:],
                                            op=ALU.mult)
                    # z += conv input rows (this core's shard of table source)
                    zin = sbm.tile([128, hid], F16, tag="zin")
                    nc.sync.dma_start(zin[:], ag[base:base + 128, :])
                    z = sbm.tile([128, hid], F32, tag="z")
                    nc.vector.tensor_tensor(z[:], zmul[:], zin[:], op=ALU.add)

                    # ---- MLP: relu(LN(z@w1+b1))@w2+b2 ----
                    zT = transpose128(z[:], "zT")
                    pu = psu.tile([128, H2], F32, tag="pu")
                    nc.tensor.matmul(pu[:], zT[:], w1_s[:, li, :],
                                     start=True, stop=True,
                                     skip_group_check=True)
                    xb = sbm.tile([128, H2], F32, tag="xb")
                    nc.vector.tensor_tensor(xb[:], pu[:],
                                            b1_s[:, li, :], op=ALU.add)
                    r = sbm.tile([128, H2], F32, tag="r")
                    layer_norm(r[:], xb[:], g1_s[:, li, :], bb1_s[:, li, :],
                               "a", srelu)
                    rT0 = transpose128(r[:, 0:128], "rT0")
                    rT1 = transpose128(r[:, 128:256], "rT1", copy_eng="vector")
                    po = pso.tile([128, hid], F32, tag="po")
                    nc.tensor.matmul(po[:], rT0[:], w2_s[:, li, 0, :],
                                     start=True, stop=False,
                                     skip_group_check=True)
                    nc.tensor.matmul(po[:], rT1[:], w2_s[:, li, 1, :],
                                     start=False, stop=True,
                                     skip_group_check=True)

                    # ---- layer epilogue: hcur = po + b2 (+ hprev) ----
                    hcur = sbm.tile([128, hid], F32, tag="hcur")
                    nc.vector.tensor_tensor(hcur[:], po[:], b2_s[:, li, :],
                                            op=ALU.add)
                    if li > 0:
                        hprev = sbm.tile([128, hid], F32, tag="hprev")
                        nc.sync.dma_start(hprev[:],
                                          h_state[base:base + 128, :])
                        nc.vector.tensor_tensor(hcur[:], hcur[:], hprev[:],
                                                op=ALU.add)
                    if li < L - 1:
                        nc.sync.dma_start(h_state[base:base + 128, :],
                                          hcur[:])
                        # z for next layer: relu(LN(h; norm[li+1]))
                        znext = sbm.tile([128, hid], F16, tag="znext")
                        layer_norm(znext[:], hcur[:], ng_s[:, li + 1, :],
                                   nb_s[:, li + 1, :], "b", srelu)
                        nc.sync.dma_start(ag_next[base:base + 128, :],
                                          znext[:])
                        if (w + 1) % ag_chunk == 0 or w == W - 1:
                            emit_ag(ag_next, next_table, w // ag_chunk)
                    else:
                        # final: relu(LN(h; norm[0])) @ lin_w + lin_b
                        fin_t = sbm.tile([128, hid], F32, tag="fin")
                        layer_norm(fin_t[:], hcur[:], ng_s[:, 0, :],
                                   nb_s[:, 0, :], "b", srelu)
                        finT = transpose128(fin_t[:], "finT")
                        pc = pso.tile([128, ncls], F32, tag="po")
                        nc.tensor.matmul(pc[:], finT[:], linw_s[:],
                                         start=True, stop=True,
                                         skip_group_check=True)
                        ow = sbm.tile([128, ncls], F32, tag="ow")
                        nc.vector.tensor_tensor(ow[:], pc[:], linb_s[:],
                                                op=ALU.add)
                        nc.sync.dma_start(out_d[base:base + 128, :], ow[:])

                if li < L - 1:
                    table = next_table
                    ag = ag_next

    nc.compile()
    return nc


# --------------------------------------------------------------------------
# entry point
# --------------------------------------------------------------------------

_CACHE = {}


def _get_program(T):
    if T not in _CACHE:
        _CACHE[T] = build_program(T)
    return _CACHE[T]


def _install_ntff_hook():
    """Bridge trn_agent_boot's ctypes NTFF profiler into antenv.axon_hooks
    (absent from this image) so run_bass_kernel_spmd(trace=True) works."""
    import types

    if "antenv.axon_hooks" in sys.modules:
        return
    try:
        sys.path.insert(0, "/root/.axon_site")
        from trn_agent_boot.trn_boot import _ntff_profile_via_ctypes

        hook = _ntff_profile_via_ctypes("/opt/axon/libaxon_pjrt.so")
    except Exception:
        hook = None
    m = types.ModuleType("antenv.axon_hooks")
    state = {"hook": hook}
    m.get_axon_ntff_profile_hook = lambda: state["hook"]
    m.set_axon_ntff_profile_hook = lambda h: state.update(hook=h)
    sys.modules["antenv.axon_hooks"] = m
    import antenv

    antenv.axon_hooks = m


def run(inputs, trace=False):
    if trace:
        _install_ntff_hook()
    T, newid, in_maps = _prep_inputs(inputs)
    nc = _get_program(T)
    res = run_bass_kernel_spmd(nc, in_maps, list(range(CORES)), trace=trace)
    full = np.concatenate([res.results[c]["out"] for c in range(CORES)],
                          axis=0)
    out = np.ascontiguousarray(full[newid])
    return out, res


def kernel(**inputs) -> np.ndarray:
    out, _ = run(inputs, trace=False)
    return out


# revision 24
# speedup vs baseline: 1.4697x; 1.4697x over previous
"""CGCNN / GENConv GNN message-passing kernel for 8 Trainium2 NeuronCores.

Strategy (dst-sharded edge parallel, load-balanced windows):
  - Host assigns nodes to 240 windows of 128 slots (30720 padded table) with
    a greedy balance on in-degree so every window receives ~2000 edges
    (T = 16 tiles of 128 edges).  Core k owns windows [k*30, (k+1)*30).
    Segment softmax/sums never cross cores.
  - Each layer: every core gathers h[src] for its edges from a replicated
    node table in its local DRAM (dma_gather, 256B rows, single_packet=False
    so the 16 SDMA engines pipeline the random HBM reads), computes
    msg = relu(h_src + ea) + eps, e = exp(t*msg), me = msg*e, and
    segment-reduces [sum e | sum me] per 128-node window with a one-hot
    matmul accumulated in PSUM.  agg = (sum me) / (sum e + eps) (algebraic
    equivalent of the reference segment softmax).
  - Node MLP / LayerNorm runs data-parallel on the core's own node shard
    (bn_stats/bn_aggr + reciprocal_approx_fast LN).
  - The updated table is AllGather'ed in two chunks per layer so the
    collective overlaps the tail windows' compute.

kernel(**inputs) takes the FULL inputs and returns the FULL [30000, 10]
output; sharding + compilation happen inside (compiled program is cached).
"""

import os
import sys

sys.path.insert(0, "/opt/trn_rl_repo")

import numpy as np

import concourse.bacc as bacc
import concourse.bass as bass
import concourse.mybir as mybir
import concourse.tile as tile
from concourse.bass_utils import run_bass_kernel_spmd
from concourse.library_config import mlp as mlp_lib

F32 = mybir.dt.float32
F16 = mybir.dt.float16
I32 = mybir.dt.int32
AF = mybir.ActivationFunctionType
ALU = mybir.AluOpType

MSG_EPS = 1e-7
SM_EPS = 1e-16
LN_EPS = 1e-5

# problem dims (hardcoded per harness contract)
N_NODES = 30000
N_EDGES = 480000
F_IN = 64
F_EDGE = 16
HID = 128
N_LAYERS = 3
N_CLASSES = 10
CORES = 8
WIN = 128
W = 30                     # windows per core
NSLOT = CORES * W * WIN    # padded node table (30720)
NPC = W * WIN              # padded nodes per core (3840)

ME_ENGINE = os.environ.get("K_ME_ENGINE", "vector")  # gpsimd | vector
SINGLE_PACKET = os.environ.get("K_SINGLE_PACKET", "0") == "1"
AG_CHUNKS = int(os.environ.get("K_AG_CHUNKS", "1"))


# --------------------------------------------------------------------------
# host-side sharding / packing
# --------------------------------------------------------------------------

def _balance_nodes(dst, n_nodes):
    """Assign nodes to CORES*W windows of WIN slots, balancing edge counts.

    Returns newid[orig_node] -> padded slot id in [0, NSLOT).
    """
    import heapq

    nwin = CORES * W
    indeg = np.bincount(dst, minlength=n_nodes).astype(np.int64)
    order = np.argsort(-indeg, kind="stable")
    heap = [(0, 0, w) for w in range(nwin)]  # (load, used, win)
    heapq.heapify(heap)
    newid = np.empty(n_nodes, np.int64)
    # process nodes in blocks of nwin for speed: pop all, assign, push back
    i = 0
    while i < n_nodes:
        k = min(nwin, len(heap), n_nodes - i)
        block = order[i:i + k]
        entries = [heapq.heappop(heap) for _ in range(k)]
        kept = []
        for j, node in enumerate(block):
            load, used, w = entries[j]
            newid[node] = w * WIN + used
            load += indeg[node]
            used += 1
            if used < WIN:
                kept.append((load, used, w))
        for e in kept:
            heapq.heappush(heap, e)
        i += k
    return newid


def _prep_edges(edge_index, edge_attr, n_nodes):
    """Permute + sort edges by dst window, pack per (core, window, tile)."""
    src = edge_index[0].astype(np.int64)
    dst = edge_index[1].astype(np.int64)
    newid = _balance_nodes(dst, n_nodes)
    srcp = newid[src]
    dstp = newid[dst]

    order = np.argsort(dstp, kind="stable")
    srcp = srcp[order]
    dstp = dstp[order]
    attr = edge_attr[order]

    gwin = dstp // WIN  # global window of each edge, non-decreasing
    counts = np.bincount(gwin, minlength=CORES * W)
    T = int(np.ceil(counts.max() / 128.0))
    T = max(T, 1)
    epw = T * 128
    starts = np.zeros(CORES * W + 1, np.int64)
    np.cumsum(counts, out=starts[1:])

    fe = attr.shape[1]
    src_pad = np.zeros((CORES, W, epw), np.int64)
    dstloc_pad = np.full((CORES, W, epw), -1.0, np.float16)
    attr_pad = np.zeros((CORES, W, epw, fe), np.float32)
    for c in range(CORES):
        for w in range(W):
            g = c * W + w
            s, e = starts[g], starts[g + 1]
            n = e - s
            src_pad[c, w, :n] = srcp[s:e]
            dstloc_pad[c, w, :n] = (dstp[s:e] - g * WIN).astype(np.float16)
            attr_pad[c, w, :n, :] = attr[s:e]

    # gather index layout: [128, W*T*8] int16, idx i of window w at
    # partition i%16 (replicated x8), column w*T*8 + i//16
    sp = src_pad.reshape(CORES, W, T * 8, 16)
    gidx16 = np.transpose(sp, (0, 3, 1, 2)).reshape(CORES, 16, W * T * 8)
    gidx = np.tile(gidx16, (1, 8, 1)).astype(np.int16)

    # dstloc: [128, W*T] f16, tile j=w*T+g column, partition = edge offset
    dl = dstloc_pad.reshape(CORES, W, T, 128)
    dstloc = np.ascontiguousarray(
        np.transpose(dl, (0, 3, 1, 2)).reshape(CORES, 128, W * T)
    )

    # attrT: [17, EPAD]: rows 0..15 features (transposed), row 16 = ones
    ap = attr_pad.reshape(CORES, W * epw, fe)
    attrT = np.empty((CORES, fe + 1, W * epw), np.float16)
    attrT[:, :fe, :] = np.transpose(ap, (0, 2, 1))
    attrT[:, fe, :] = 1.0
    return T, newid, gidx, dstloc, np.ascontiguousarray(attrT)


def _prep_inputs(inputs):
    """Build the 8 per-core input maps (and shared weight arrays)."""
    x = np.asarray(inputs["x"], np.float32)
    edge_attr = np.asarray(inputs["edge_attr"], np.float32)
    edge_index = np.asarray(inputs["edge_index"])
    n_nodes, fin = x.shape

    T, newid, gidx, dstloc, attrT = _prep_edges(edge_index, edge_attr, n_nodes)

    L = int(np.asarray(inputs["t"]).shape[0])
    hid = np.asarray(inputs["node_enc_w"]).shape[1]
    H2 = 2 * hid

    # permuted node features: xp[newid[i]] = x[i]; empty slots zero
    xp = np.zeros((NSLOT, fin), np.float32)
    xp[newid] = x
    xs = xp.reshape(CORES, NPC, fin)
    xT = np.empty((CORES, fin + 1, NPC), np.float32)
    xT[:, :fin, :] = np.transpose(xs, (0, 2, 1))
    xT[:, fin, :] = 1.0

    wnode = np.concatenate(
        [np.asarray(inputs["node_enc_w"], np.float32),
         np.asarray(inputs["node_enc_b"], np.float32)[None, :]], 0
    )
    wenc = np.concatenate(
        [np.asarray(inputs["edge_enc_w"], np.float32),
         np.asarray(inputs["edge_enc_b"], np.float32)[None, :]], 0
    ).astype(np.float16)
    w1 = np.ascontiguousarray(inputs["mlp1_w"], np.float16)      # [L,H,2H]
    w2 = np.ascontiguousarray(inputs["mlp2_w"], np.float16)      # [L,2H,H]

    def bc(a, n):
        return np.ascontiguousarray(
            np.broadcast_to(np.asarray(a, np.float16)[:, None, :], (L, 128, n)))

    b1bc = bc(inputs["mlp1_b"], H2)
    b2bc = bc(inputs["mlp2_b"], hid)
    g1bc = bc(inputs["mlp_ln_g"], H2)
    bb1bc = bc(inputs["mlp_ln_b"], H2)
    ngbc = bc(inputs["norm_g"], hid)
    nbbc = bc(inputs["norm_b"], hid)
    tcol = np.ascontiguousarray(
        np.broadcast_to(np.asarray(inputs["t"], np.float32)[None, :], (128, L)))
    linw = np.ascontiguousarray(inputs["lin_w"], np.float16)
    linbbc = np.ascontiguousarray(
        np.broadcast_to(np.asarray(inputs["lin_b"], np.float16)[None, :],
                        (128, N_CLASSES)))
    iota = np.ascontiguousarray(
        np.broadcast_to(np.arange(128, dtype=np.float16)[None, :], (128, 128)))
    ident = np.eye(128, dtype=np.float32)
    identh = np.eye(128, dtype=np.float16)

    shared = dict(wnode=wnode, wench=wenc, w1h=w1, w2h=w2, b1bc=b1bc,
                  b2bc=b2bc, g1bc=g1bc, bb1bc=bb1bc, ngbc=ngbc, nbbc=nbbc,
                  tcol=tcol, linwh=linw, linbbc=linbbc, iota=iota,
                  ident=ident, identh=identh)
    in_maps = []
    for c in range(CORES):
        m = dict(shared)
        m["xT"] = np.ascontiguousarray(xT[c])
        m["attrT"] = attrT[c]
        m["gidx"] = np.ascontiguousarray(gidx[c])
        m["dstloc"] = dstloc[c]
        in_maps.append(m)
    return T, newid, in_maps


# --------------------------------------------------------------------------
# device program
# --------------------------------------------------------------------------

def build_program(T, fin=F_IN, fe=F_EDGE, hid=HID, L=N_LAYERS, ncls=N_CLASSES):
    EPAD = W * T * 128
    H2 = 2 * hid
    npc = NPC
    nslot = NSLOT
    nc = bacc.Bacc("TRN2", target_bir_lowering=False, debug=False,
                   num_devices=CORES, dynamic_dma_scratch_size=131072,
                   num_swdge_queues=4)

    d = nc.dram_tensor
    xT_d = d("xT", [fin + 1, npc], F32, kind="ExternalInput")
    attrT_d = d("attrT", [fe + 1, EPAD], F16, kind="ExternalInput")
    gidx_d = d("gidx", [128, W * T * 8], mybir.dt.int16, kind="ExternalInput")
    dstloc_d = d("dstloc", [128, W * T], F16, kind="ExternalInput")
    wnode_d = d("wnode", [fin + 1, hid], F32, kind="ExternalInput")
    wenc_d = d("wench", [fe + 1, hid], F16, kind="ExternalInput")
    w1_d = d("w1h", [L, hid, H2], F16, kind="ExternalInput")
    w2_d = d("w2h", [L, H2, hid], F16, kind="ExternalInput")
    b1bc_d = d("b1bc", [L, 128, H2], F16, kind="ExternalInput")
    b2bc_d = d("b2bc", [L, 128, hid], F16, kind="ExternalInput")
    g1bc_d = d("g1bc", [L, 128, H2], F16, kind="ExternalInput")
    bb1bc_d = d("bb1bc", [L, 128, H2], F16, kind="ExternalInput")
    ngbc_d = d("ngbc", [L, 128, hid], F16, kind="ExternalInput")
    nbbc_d = d("nbbc", [L, 128, hid], F16, kind="ExternalInput")
    tcol_d = d("tcol", [128, L], F32, kind="ExternalInput")
    linw_d = d("linwh", [hid, ncls], F16, kind="ExternalInput")
    linbbc_d = d("linbbc", [128, ncls], F16, kind="ExternalInput")
    iota_d = d("iota", [128, 128], F16, kind="ExternalInput")
    ident_d = d("ident", [128, 128], F32, kind="ExternalInput")
    identh_d = d("identh", [128, 128], F16, kind="ExternalInput")
    out_d = d("out", [npc, ncls], F32, kind="ExternalOutput")

    rg = [list(range(CORES))]
    NQ = (T + 3) // 4          # quads of <=4 tiles per window
    NG = (T + 7) // 8          # gather calls of <=8 tiles per window
    ag_chunk = (W + AG_CHUNKS - 1) // AG_CHUNKS

    with tile.TileContext(nc) as tc:
        nc.gpsimd.load_library(mlp_lib)
        with (
            tc.tile_pool(name="const", bufs=1) as cp,
            tc.tile_pool(name="sbw", bufs=2) as sbw,       # window tiles
            tc.tile_pool(name="sbm", bufs=2) as sbm,       # MLP scratch
            tc.tile_pool(name="psq", bufs=2, space="PSUM") as psq,
            tc.tile_pool(name="psa", bufs=2, space="PSUM") as psa,
            tc.tile_pool(name="psu", bufs=2, space="PSUM") as psu,
            tc.tile_pool(name="pst", bufs=1, space="PSUM") as pst,
            tc.tile_pool(name="pso", bufs=1, space="PSUM") as pso,
            tc.tile_pool(name="dram", bufs=2, space="DRAM") as dp,
            tc.tile_pool(name="dram1", bufs=1, space="DRAM") as dp1,
        ):
            # ---------------- constants / weights to SBUF ----------------
            def load(name, dram_ap, shape, rearr=None, dt_=F32, **kw):
                t = cp.tile(shape, dt_, tag=name)
                src = dram_ap if rearr is None else dram_ap.rearrange(rearr, **kw)
                nc.sync.dma_start(t[:], src)
                return t

            wnode_s = load("wnode", wnode_d[:, :], [fin + 1, hid])
            wenc_s = load("wenc", wenc_d[:, :], [fe + 1, hid], dt_=F16)
            w1_s = load("w1", w1_d[:, :, :], [hid, L, H2], "l k n -> k l n",
                        dt_=F16)
            w2_s = load("w2", w2_d[:, :, :], [128, L, 2, hid],
                        "l (h k) n -> k l h n", h=2, dt_=F16)
            b1_s = load("b1", b1bc_d[:, :, :], [128, L, H2], "l p n -> p l n",
                        dt_=F16)
            b2_s = load("b2", b2bc_d[:, :, :], [128, L, hid], "l p n -> p l n",
                        dt_=F16)
            g1_s = load("g1", g1bc_d[:, :, :], [128, L, H2], "l p n -> p l n",
                        dt_=F16)
            bb1_s = load("bb1", bb1bc_d[:, :, :], [128, L, H2],
                         "l p n -> p l n", dt_=F16)
            ng_s = load("ng", ngbc_d[:, :, :], [128, L, hid], "l p n -> p l n",
                        dt_=F16)
            nb_s = load("nb", nbbc_d[:, :, :], [128, L, hid], "l p n -> p l n",
                        dt_=F16)
            tcol_s = load("tcol", tcol_d[:, :], [128, L])
            linw_s = load("linw", linw_d[:, :], [hid, ncls], dt_=F16)
            linb_s = load("linb", linbbc_d[:, :], [128, ncls], dt_=F16)
            iota_s = load("iota", iota_d[:, :], [128, 128], dt_=F16)
            ident_s = load("ident", ident_d[:, :], [128, 128])
            identh_s = load("identh", identh_d[:, :], [128, 128], dt_=F16)
            dstloc_s = load("dstloc", dstloc_d[:, :], [128, W * T], dt_=F16)
            gidx_s = cp.tile([128, W * T * 8], mybir.dt.int16, tag="gidx")
            nc.sync.dma_start(gidx_s[:], gidx_d[:, :])

            ln_col = cp.tile([128, 1], F32, tag="lnc")
            nc.vector.memset(ln_col[:], LN_EPS)
            eps_col = cp.tile([128, 1], F32, tag="epsc")
            nc.vector.memset(eps_col[:], MSG_EPS)

            h_state = dp1.tile([npc, hid], F32)
            S_dram = dp1.tile([128, W * T * 128], F16)

            # ---------------- helpers ----------------
            def layer_norm(dst_ap, src_ap, g_ap, b_ap, tag, relu_eng):
                """dst <- relu(LN(src) * g + b).  src must be SBUF/PSUM f32."""
                st6 = sbm.tile([128, 6], F32, tag="lnst" + tag)
                nc.vector.bn_stats(st6[:], src_ap)
                ag2 = sbm.tile([128, 2], F32, tag="lnag" + tag)
                nc.vector.bn_aggr(ag2[:], st6[:])
                # rstd = exp(-0.5*ln(var+eps)); Ln+Exp share one ACT table
                # set (natural_log_exp_and_others) so no table reloads.
                lv = sbm.tile([128, 1], F32, tag="lnlv" + tag)
                nc.scalar.activation(lv[:], ag2[:, 1:2], AF.Ln, bias=ln_col[:])
                rstd = sbm.tile([128, 1], F32, tag="lnrs" + tag)
                nc.scalar.activation(rstd[:], lv[:], AF.Exp, scale=-0.5)
                nfeat = src_ap.shape[-1]
                y = sbm.tile([128, nfeat], F32, tag="lny" + tag)
                nc.vector.tensor_scalar(y[:], src_ap, ag2[:, 0:1], rstd[:],
                                        op0=ALU.subtract, op1=ALU.mult)
                nc.vector.tensor_tensor(y[:], y[:], g_ap, op=ALU.mult)
                nc.vector.tensor_tensor(y[:], y[:], b_ap, op=ALU.add)
                relu_eng(dst_ap, y[:])

            def srelu(dst_ap, src_ap):
                nc.scalar.activation(dst_ap, src_ap, AF.Relu)

            def vrelu(dst_ap, src_ap):
                nc.vector.tensor_scalar_max(dst_ap, src_ap, 0.0)

            def transpose128(src_ap, tag, copy_eng="scalar"):
                """PE transpose [128,128] f32 -> SBUF f16."""
                pt = pst.tile([128, 128], F32, tag="pt")
                nc.tensor.transpose(pt[:], src_ap, ident_s[:])
                st = sbm.tile([128, 128], F16, tag=tag)
                if copy_eng == "scalar":
                    nc.scalar.activation(st[:], pt[:], AF.Copy)
                else:
                    nc.vector.tensor_copy(st[:], pt[:])
                return st

            # ---------------- encode phase: h0 = x @ wnode ----------------
            def new_ag(nm):
                return dp.tile([npc, hid], F16, tag="ag", name=nm)

            def new_table(nm):
                return dp.tile([nslot, hid], F16, tag="table",
                               addr_space="Shared", name=nm)

            def emit_ag(agp, table, chunk):
                base = chunk * ag_chunk * 128
                rows = min(ag_chunk * 128, npc - base)
                tview = table[:, :].rearrange("(c n) h -> c n h", c=CORES)
                nc.gpsimd.collective_compute(
                    "AllGather", ALU.bypass,
                    ins=[agp[base:base + rows, :].opt()],
                    outs=[tview[:, base:base + rows, :].opt()],
                    replica_groups=rg)

            ag = new_ag("age")
            table = new_table("tbl0")
            for w in range(W):
                base = w * 128
                xts = sbm.tile([fin + 1, 128], F32, tag="xts")
                nc.sync.dma_start(xts[:], xT_d[:, base:base + 128])
                ph = pso.tile([128, hid], F32, tag="po")
                nc.tensor.matmul(ph[:], xts[:], wnode_s[:],
                                 start=True, stop=True)
                h0 = sbm.tile([128, hid], F16, tag="h0")
                nc.vector.tensor_copy(h0[:], ph[:])
                nc.sync.dma_start(ag[base:base + 128, :], h0[:])
                if (w + 1) % ag_chunk == 0 or w == W - 1:
                    emit_ag(ag, table, w // ag_chunk)

            # ---------------- conv layers ----------------
            for li in range(L):
                ag_next = new_ag(f"agn{li}") if li < L - 1 else None
                next_table = new_table(f"tbl{li + 1}") if li < L - 1 else None
                for w in range(W):
                    base = w * 128
                    jw = w * T

                    # gather h[src] for this window: [128, T, 128]
                    hsrc = sbw.tile([128, T, 128], F16, tag="hsrc", bufs=3)
                    for qi in range(NG):
                        c0 = qi * 8
                        ct = min(8, T - c0)
                        nc.gpsimd.dma_gather(
                            hsrc[:, c0:c0 + ct, :], table[:, :],
                            gidx_s[:, (jw + c0) * 8:(jw + c0 + ct) * 8],
                            ct * 128, ct * 128, hid,
                            queue_num=(w * NG + qi) % 4,
                            single_packet=SINGLE_PACKET)
                    attrs = sbw.tile([fe + 1, T, 128], F16, tag="attrs")
                    nc.sync.dma_start(
                        attrs[:], attrT_d[:, jw * 128:(jw + T) * 128])

                    msg = sbw.tile([128, T, 128], F16, tag="msg")
                    em = sbw.tile([128, 2, T, 128], F16, tag="em", bufs=2)
                    # quads: ea matmul + gathered-h add (identity matmul)
                    for q in range(NQ):
                        q0 = q * 4
                        qs = min(4, T - q0)
                        pq = psq.tile([128, 4, 128], F32, tag="pq")
                        for j in range(qs):
                            nc.tensor.matmul(
                                pq[:, j, :], attrs[:, q0 + j, :], wenc_s[:],
                                start=(j == 0), stop=False,
                                skip_group_check=True)
                        nc.tensor.matmul(
                            pq[:, :qs, :], identh_s[:],
                            hsrc[:, q0:q0 + qs, :],
                            start=False, stop=True, skip_group_check=True)
                        # msg = relu(ea + h_src + eps)
                        nc.scalar.activation(
                            msg[:, q0:q0 + qs, :], pq[:, :qs, :], AF.Relu,
                            bias=eps_col[:])
                    # e = exp(t * msg) ; me = msg * e
                    nc.scalar.activation(
                        em[:, 0, :, :], msg[:], AF.Exp,
                        scale=tcol_s[:, li:li + 1])
                    me_eng = nc.gpsimd if ME_ENGINE == "gpsimd" else nc.vector
                    me_eng.tensor_tensor(
                        em[:, 1, :, :], msg[:], em[:, 0, :, :], op=ALU.mult)
                    # one-hot S for the whole window (all-f16); static across
                    # layers: computed in layer 0, streamed back afterwards
                    S = sbw.tile([128, T, 128], F16, tag="S")
                    if li == 0:
                        iota_b = iota_s[:].rearrange(
                            "p (o f) -> p o f", o=1).broadcast_to([128, T, 128])
                        dl_b = dstloc_s[:, jw:jw + T].rearrange(
                            "p (t o) -> p t o", o=1).broadcast_to([128, T, 128])
                        nc.vector.tensor_tensor(S[:], iota_b, dl_b,
                                                op=ALU.is_equal)
                        nc.sync.dma_start(
                            S_dram[:, jw * 128:(jw + T) * 128],
                            S[:].rearrange("p t f -> p (t f)"))
                    else:
                        nc.sync.dma_start(
                            S[:].rearrange("p t f -> p (t f)"),
                            S_dram[:, jw * 128:(jw + T) * 128])
                    # segment accumulate [sum e | sum me] -> [128, 256] psum
                    acc = psa.tile([128, 2, hid], F32, tag="acc")
                    for g in range(T):
                        nc.tensor.matmul(
                            acc[:, :, :], S[:, g, :], em[:, :, g, :],
                            start=(g == 0), stop=(g == T - 1))

                    # agg = (sum me) / (sum e + eps)
                    sep = sbm.tile([128, hid], F32, tag="sep")
                    nc.vector.tensor_scalar_add(sep[:], acc[:, 0, :], SM_EPS)
                    rcse = sbm.tile([128, hid], F32, tag="rcse")
                    nc.vector.reciprocal_approx_fast(rcse[:], sep[:])
                    zmul = sbm.tile([128, hid], F32, tag="zmul")
                    nc.vector.tensor_tensor(zmul[:], acc[:, 1, :], rcse[:],
                                            op=ALU.mult)
                    # z += conv input rows (this core's shard of table source)
                    zin = sbm.tile([128, hid], F16, tag="zin")
                    nc.sync.dma_start(zin[:], ag[base:base + 128, :])
                    z = sbm.tile([128, hid], F32, tag="z")
                    nc.vector.tensor_tensor(z[:], zmul[:], zin[:], op=ALU.add)

                    # ---- MLP: relu(LN(z@w1+b1))@w2+b2 ----
                    zT = transpose128(z[:], "zT")
                    pu = psu.tile([128, H2], F32, tag="pu")
                    nc.tensor.matmul(pu[:], zT[:], w1_s[:, li, :],
                                     start=True, stop=True,
                                     skip_group_check=True)
                    xb = sbm.tile([128, H2], F32, tag="xb")
                    nc.vector.tensor_tensor(xb[:], pu[:],
                                            b1_s[:, li, :], op=ALU.add)
                    r = sbm.tile([128, H2], F32, tag="r")
                    layer_norm(r[:], xb[:], g1_s[:, li, :], bb1_s[:, li, :],
                               "a", srelu)
                    rT0 = transpose128(r[:, 0:128], "rT0")
                    rT1 = transpose128(r[:, 128:256], "rT1", copy_eng="vector")
                    po = pso.tile([128, hid], F32, tag="po")
                    nc.tensor.matmul(po[:], rT0[:], w2_s[:, li, 0, :],
                                     start=True, stop=False,
                                     skip_group_check=True)
                    nc.tensor.matmul(po[:], rT1[:], w2_s[:, li, 1, :],
                                     start=False, stop=True,
                                     skip_group_check=True)

                    # ---- layer epilogue: hcur = po + b2 (+ hprev) ----
                    hcur = sbm.tile([128, hid], F32, tag="hcur")
                    nc.vector.tensor_tensor(hcur[:], po[:], b2_s[:, li, :],
                                            op=ALU.add)
                    if li > 0:
                        hprev = sbm.tile([128, hid], F32, tag="hprev")
                        nc.sync.dma_start(hprev[:],
                                          h_state[base:base + 128, :])
                        nc.vector.tensor_tensor(hcur[:], hcur[:], hprev[:],
                                                op=ALU.add)
                    if li < L - 1:
                        nc.sync.dma_start(h_state[base:base + 128, :],
                                          hcur[:])
                        # z for next layer: relu(LN(h; norm[li+1]))
                        znext = sbm.tile([128, hid], F16, tag="znext")
                        layer_norm(znext[:], hcur[:], ng_s[:, li + 1, :],
                                   nb_s[:, li + 1, :], "b", srelu)
                        nc.sync.dma_start(ag_next[base:base + 128, :],
                                          znext[:])
                        if (w + 1) % ag_chunk == 0 or w == W - 1:
                            emit_ag(ag_next, next_table, w // ag_chunk)
                    else:
                        # final: relu(LN(h; norm[0])) @ lin_w + lin_b
                        fin_t = sbm.tile([128, hid], F32, tag="fin")
                        layer_norm(fin_t[:], hcur[:], ng_s[:, 0, :],
                                   nb_s[:, 0, :], "b", srelu)
                        finT = transpose128(fin_t[:], "finT")
                        pc = pso.tile([128, ncls], F32, tag="po")
                        nc.tensor.matmul(pc[:], finT[:], linw_s[:],
                                         start=True, stop=True,
                                         skip_group_check=True)
                        ow = sbm.tile([128, ncls], F32, tag="ow")
                        nc.vector.tensor_tensor(ow[:], pc[:], linb_s[:],
                                                op=ALU.add)
                        nc.sync.dma_start(out_d[base:base + 128, :], ow[:])

                if li < L - 1:
                    table = next_table
                    ag = ag_next

    nc.compile()
    return nc


# --------------------------------------------------------------------------
# entry point
# --------------------------------------------------------------------------

_CACHE = {}


def _get_program(T):
    if T not in _CACHE:
        _CACHE[T] = build_program(T)
    return _CACHE[T]


def _install_ntff_hook():
    """Bridge trn_agent_boot's ctypes NTFF profiler into antenv.axon_hooks
    (absent from this image) so run_bass_kernel_spmd(trace=True) works."""
    import types

    if "antenv.axon_hooks" in sys.modules:
        return
    try:
        sys.path.insert(0, "/root/.axon_site")
        from trn_agent_boot.trn_boot import _ntff_profile_via_ctypes

        hook = _ntff_profile_via_ctypes("/opt/axon/libaxon_pjrt.so")
    except Exception:
        hook = None
    m = types.ModuleType("antenv.axon_hooks")
    state = {"hook": hook}
    m.get_axon_ntff_profile_hook = lambda: state["hook"]
    m.set_axon_ntff_profile_hook = lambda h: state.update(hook=h)
    sys.modules["antenv.axon_hooks"] = m
    import antenv

    antenv.axon_hooks = m


def run(inputs, trace=False):
    if trace:
        _install_ntff_hook()
    T, newid, in_maps = _prep_inputs(inputs)
    nc = _get_program(T)
    res = run_bass_kernel_spmd(nc, in_maps, list(range(CORES)), trace=trace)
    full = np.concatenate([res.results[c]["out"] for c in range(CORES)],
                          axis=0)
    out = np.ascontiguousarray(full[newid])
    return out, res


def kernel(**inputs) -> np.ndarray:
    out, _ = run(inputs, trace=False)
    return out


# revision 25
# speedup vs baseline: 1.6544x; 1.1256x over previous
"""CGCNN / GENConv GNN message-passing kernel for 8 Trainium2 NeuronCores.

Strategy (dst-sharded edge parallel, load-balanced windows):
  - Host assigns nodes to 240 windows of 128 slots (30720 padded table) with
    a greedy balance on in-degree so every window receives ~2000 edges
    (T = 16 tiles of 128 edges).  Core k owns windows [k*30, (k+1)*30).
    Segment softmax/sums never cross cores.
  - Each layer: every core gathers h[src] for its edges from a replicated
    node table in its local DRAM (dma_gather, 256B rows, single_packet=False
    so the 16 SDMA engines pipeline the random HBM reads), computes
    msg = relu(h_src + ea) + eps, e = exp(t*msg), me = msg*e, and
    segment-reduces [sum e | sum me] per 128-node window with a one-hot
    matmul accumulated in PSUM.  agg = (sum me) / (sum e + eps) (algebraic
    equivalent of the reference segment softmax).
  - Node MLP / LayerNorm runs data-parallel on the core's own node shard
    (bn_stats/bn_aggr + reciprocal_approx_fast LN).
  - The updated table is AllGather'ed in two chunks per layer so the
    collective overlaps the tail windows' compute.

kernel(**inputs) takes the FULL inputs and returns the FULL [30000, 10]
output; sharding + compilation happen inside (compiled program is cached).
"""

import os
import sys

sys.path.insert(0, "/opt/trn_rl_repo")

import numpy as np

import concourse.bacc as bacc
import concourse.bass as bass
import concourse.mybir as mybir
import concourse.tile as tile
from concourse.bass_utils import run_bass_kernel_spmd
from concourse.library_config import mlp as mlp_lib

F32 = mybir.dt.float32
F16 = mybir.dt.float16
I32 = mybir.dt.int32
AF = mybir.ActivationFunctionType
ALU = mybir.AluOpType

MSG_EPS = 1e-7
SM_EPS = 1e-16
LN_EPS = 1e-5

# problem dims (hardcoded per harness contract)
N_NODES = 30000
N_EDGES = 480000
F_IN = 64
F_EDGE = 16
HID = 128
N_LAYERS = 3
N_CLASSES = 10
CORES = 8
WIN = 128
W = 30                     # windows per core
NSLOT = CORES * W * WIN    # padded node table (30720)
NPC = W * WIN              # padded nodes per core (3840)

ME_ENGINE = os.environ.get("K_ME_ENGINE", "vector")  # gpsimd | vector
SINGLE_PACKET = os.environ.get("K_SINGLE_PACKET", "0") == "1"
AG_CHUNKS = int(os.environ.get("K_AG_CHUNKS", "1"))


# --------------------------------------------------------------------------
# host-side sharding / packing
# --------------------------------------------------------------------------

def _balance_nodes(dst, n_nodes):
    """Assign nodes to CORES*W windows of WIN slots, balancing edge counts.

    Returns newid[orig_node] -> padded slot id in [0, NSLOT).
    """
    import heapq

    nwin = CORES * W
    indeg = np.bincount(dst, minlength=n_nodes).astype(np.int64)
    order = np.argsort(-indeg, kind="stable")
    heap = [(0, 0, w) for w in range(nwin)]  # (load, used, win)
    heapq.heapify(heap)
    newid = np.empty(n_nodes, np.int64)
    # process nodes in blocks of nwin for speed: pop all, assign, push back
    i = 0
    while i < n_nodes:
        k = min(nwin, len(heap), n_nodes - i)
        block = order[i:i + k]
        entries = [heapq.heappop(heap) for _ in range(k)]
        kept = []
        for j, node in enumerate(block):
            load, used, w = entries[j]
            newid[node] = w * WIN + used
            load += indeg[node]
            used += 1
            if used < WIN:
                kept.append((load, used, w))
        for e in kept:
            heapq.heappush(heap, e)
        i += k
    return newid


def _prep_edges(edge_index, edge_attr, n_nodes):
    """Permute + sort edges by dst window, pack per (core, window, tile)."""
    src = edge_index[0].astype(np.int64)
    dst = edge_index[1].astype(np.int64)
    newid = _balance_nodes(dst, n_nodes)
    srcp = newid[src]
    dstp = newid[dst]

    order = np.argsort(dstp, kind="stable")
    srcp = srcp[order]
    dstp = dstp[order]
    attr = edge_attr[order]

    gwin = dstp // WIN  # global window of each edge, non-decreasing
    counts = np.bincount(gwin, minlength=CORES * W)
    T = int(np.ceil(counts.max() / 128.0))
    T = max(T, 1)
    epw = T * 128
    starts = np.zeros(CORES * W + 1, np.int64)
    np.cumsum(counts, out=starts[1:])

    fe = attr.shape[1]
    src_pad = np.zeros((CORES, W, epw), np.int64)
    dstloc_pad = np.full((CORES, W, epw), -1.0, np.float16)
    attr_pad = np.zeros((CORES, W, epw, fe), np.float32)
    for c in range(CORES):
        for w in range(W):
            g = c * W + w
            s, e = starts[g], starts[g + 1]
            n = e - s
            src_pad[c, w, :n] = srcp[s:e]
            dstloc_pad[c, w, :n] = (dstp[s:e] - g * WIN).astype(np.float16)
            attr_pad[c, w, :n, :] = attr[s:e]

    # gather index layout: [128, W*T*8] int16, idx i of window w at
    # partition i%16 (replicated x8), column w*T*8 + i//16
    sp = src_pad.reshape(CORES, W, T * 8, 16)
    gidx16 = np.transpose(sp, (0, 3, 1, 2)).reshape(CORES, 16, W * T * 8)
    gidx = np.tile(gidx16, (1, 8, 1)).astype(np.int16)

    # dstloc: [128, W*T] f16, tile j=w*T+g column, partition = edge offset
    dl = dstloc_pad.reshape(CORES, W, T, 128)
    dstloc = np.ascontiguousarray(
        np.transpose(dl, (0, 3, 1, 2)).reshape(CORES, 128, W * T)
    )

    # attrT: [17, EPAD]: rows 0..15 features (transposed), row 16 = ones
    ap = attr_pad.reshape(CORES, W * epw, fe)
    attrT = np.empty((CORES, fe + 1, W * epw), np.float16)
    attrT[:, :fe, :] = np.transpose(ap, (0, 2, 1))
    attrT[:, fe, :] = 1.0
    return T, newid, gidx, dstloc, np.ascontiguousarray(attrT)


def _prep_inputs(inputs):
    """Build the 8 per-core input maps (and shared weight arrays)."""
    x = np.asarray(inputs["x"], np.float32)
    edge_attr = np.asarray(inputs["edge_attr"], np.float32)
    edge_index = np.asarray(inputs["edge_index"])
    n_nodes, fin = x.shape

    T, newid, gidx, dstloc, attrT = _prep_edges(edge_index, edge_attr, n_nodes)

    L = int(np.asarray(inputs["t"]).shape[0])
    hid = np.asarray(inputs["node_enc_w"]).shape[1]
    H2 = 2 * hid

    # permuted node features: xp[newid[i]] = x[i]; empty slots zero
    xp = np.zeros((NSLOT, fin), np.float32)
    xp[newid] = x
    xs = xp.reshape(CORES, NPC, fin)
    xT = np.empty((CORES, fin + 1, NPC), np.float32)
    xT[:, :fin, :] = np.transpose(xs, (0, 2, 1))
    xT[:, fin, :] = 1.0

    wnode = np.concatenate(
        [np.asarray(inputs["node_enc_w"], np.float32),
         np.asarray(inputs["node_enc_b"], np.float32)[None, :]], 0
    )
    wenc = np.concatenate(
        [np.asarray(inputs["edge_enc_w"], np.float32),
         np.asarray(inputs["edge_enc_b"], np.float32)[None, :]], 0
    ).astype(np.float16)
    w1 = np.ascontiguousarray(inputs["mlp1_w"], np.float16)      # [L,H,2H]
    w2 = np.ascontiguousarray(inputs["mlp2_w"], np.float16)      # [L,2H,H]

    def bc(a, n):
        return np.ascontiguousarray(
            np.broadcast_to(np.asarray(a, np.float16)[:, None, :], (L, 128, n)))

    b1bc = bc(inputs["mlp1_b"], H2)
    b2bc = bc(inputs["mlp2_b"], hid)
    g1bc = bc(inputs["mlp_ln_g"], H2)
    bb1bc = bc(inputs["mlp_ln_b"], H2)
    ngbc = bc(inputs["norm_g"], hid)
    nbbc = bc(inputs["norm_b"], hid)
    tcol = np.ascontiguousarray(
        np.broadcast_to(np.asarray(inputs["t"], np.float32)[None, :], (128, L)))
    linw = np.ascontiguousarray(inputs["lin_w"], np.float16)
    linbbc = np.ascontiguousarray(
        np.broadcast_to(np.asarray(inputs["lin_b"], np.float16)[None, :],
                        (128, N_CLASSES)))
    iota = np.ascontiguousarray(
        np.broadcast_to(np.arange(128, dtype=np.float16)[None, :], (128, 128)))
    ident = np.eye(128, dtype=np.float32)
    identh = np.eye(128, dtype=np.float16)

    shared = dict(wnode=wnode, wench=wenc, w1h=w1, w2h=w2, b1bc=b1bc,
                  b2bc=b2bc, g1bc=g1bc, bb1bc=bb1bc, ngbc=ngbc, nbbc=nbbc,
                  tcol=tcol, linwh=linw, linbbc=linbbc, iota=iota,
                  ident=ident, identh=identh)
    in_maps = []
    for c in range(CORES):
        m = dict(shared)
        m["xT"] = np.ascontiguousarray(xT[c])
        m["attrT"] = attrT[c]
        m["gidx"] = np.ascontiguousarray(gidx[c])
        m["dstloc"] = dstloc[c]
        in_maps.append(m)
    return T, newid, in_maps


# --------------------------------------------------------------------------
# device program
# --------------------------------------------------------------------------

def build_program(T, fin=F_IN, fe=F_EDGE, hid=HID, L=N_LAYERS, ncls=N_CLASSES):
    EPAD = W * T * 128
    H2 = 2 * hid
    npc = NPC
    nslot = NSLOT
    nc = bacc.Bacc("TRN2", target_bir_lowering=False, debug=False,
                   num_devices=CORES, dynamic_dma_scratch_size=131072,
                   num_swdge_queues=4)

    d = nc.dram_tensor
    xT_d = d("xT", [fin + 1, npc], F32, kind="ExternalInput")
    attrT_d = d("attrT", [fe + 1, EPAD], F16, kind="ExternalInput")
    gidx_d = d("gidx", [128, W * T * 8], mybir.dt.int16, kind="ExternalInput")
    dstloc_d = d("dstloc", [128, W * T], F16, kind="ExternalInput")
    wnode_d = d("wnode", [fin + 1, hid], F32, kind="ExternalInput")
    wenc_d = d("wench", [fe + 1, hid], F16, kind="ExternalInput")
    w1_d = d("w1h", [L, hid, H2], F16, kind="ExternalInput")
    w2_d = d("w2h", [L, H2, hid], F16, kind="ExternalInput")
    b1bc_d = d("b1bc", [L, 128, H2], F16, kind="ExternalInput")
    b2bc_d = d("b2bc", [L, 128, hid], F16, kind="ExternalInput")
    g1bc_d = d("g1bc", [L, 128, H2], F16, kind="ExternalInput")
    bb1bc_d = d("bb1bc", [L, 128, H2], F16, kind="ExternalInput")
    ngbc_d = d("ngbc", [L, 128, hid], F16, kind="ExternalInput")
    nbbc_d = d("nbbc", [L, 128, hid], F16, kind="ExternalInput")
    tcol_d = d("tcol", [128, L], F32, kind="ExternalInput")
    linw_d = d("linwh", [hid, ncls], F16, kind="ExternalInput")
    linbbc_d = d("linbbc", [128, ncls], F16, kind="ExternalInput")
    iota_d = d("iota", [128, 128], F16, kind="ExternalInput")
    ident_d = d("ident", [128, 128], F32, kind="ExternalInput")
    identh_d = d("identh", [128, 128], F16, kind="ExternalInput")
    out_d = d("out", [npc, ncls], F32, kind="ExternalOutput")

    rg = [list(range(CORES))]
    NQ = (T + 3) // 4          # quads of <=4 tiles per window
    NG = (T + 7) // 8          # gather calls of <=8 tiles per window
    ag_chunk = (W + AG_CHUNKS - 1) // AG_CHUNKS

    with tile.TileContext(nc) as tc:
        nc.gpsimd.load_library(mlp_lib)
        with (
            tc.tile_pool(name="const", bufs=1) as cp,
            tc.tile_pool(name="sbw", bufs=2) as sbw,       # window tiles
            tc.tile_pool(name="sbm", bufs=2) as sbm,       # MLP scratch
            tc.tile_pool(name="psq", bufs=2, space="PSUM") as psq,
            tc.tile_pool(name="psa", bufs=2, space="PSUM") as psa,
            tc.tile_pool(name="psu", bufs=2, space="PSUM") as psu,
            tc.tile_pool(name="pst", bufs=1, space="PSUM") as pst,
            tc.tile_pool(name="pso", bufs=1, space="PSUM") as pso,
            tc.tile_pool(name="dram", bufs=2, space="DRAM") as dp,
            tc.tile_pool(name="dram1", bufs=1, space="DRAM") as dp1,
        ):
            # ---------------- constants / weights to SBUF ----------------
            def load(name, dram_ap, shape, rearr=None, dt_=F32, **kw):
                t = cp.tile(shape, dt_, tag=name)
                src = dram_ap if rearr is None else dram_ap.rearrange(rearr, **kw)
                nc.sync.dma_start(t[:], src)
                return t

            wnode_s = load("wnode", wnode_d[:, :], [fin + 1, hid])
            wenc_s = load("wenc", wenc_d[:, :], [fe + 1, hid], dt_=F16)
            w1_s = load("w1", w1_d[:, :, :], [hid, L, H2], "l k n -> k l n",
                        dt_=F16)
            w2_s = load("w2", w2_d[:, :, :], [128, L, 2, hid],
                        "l (h k) n -> k l h n", h=2, dt_=F16)
            b1_s = load("b1", b1bc_d[:, :, :], [128, L, H2], "l p n -> p l n",
                        dt_=F16)
            b2_s = load("b2", b2bc_d[:, :, :], [128, L, hid], "l p n -> p l n",
                        dt_=F16)
            g1_s = load("g1", g1bc_d[:, :, :], [128, L, H2], "l p n -> p l n",
                        dt_=F16)
            bb1_s = load("bb1", bb1bc_d[:, :, :], [128, L, H2],
                         "l p n -> p l n", dt_=F16)
            ng_s = load("ng", ngbc_d[:, :, :], [128, L, hid], "l p n -> p l n",
                        dt_=F16)
            nb_s = load("nb", nbbc_d[:, :, :], [128, L, hid], "l p n -> p l n",
                        dt_=F16)
            tcol_s = load("tcol", tcol_d[:, :], [128, L])
            linw_s = load("linw", linw_d[:, :], [hid, ncls], dt_=F16)
            linb_s = load("linb", linbbc_d[:, :], [128, ncls], dt_=F16)
            iota_s = load("iota", iota_d[:, :], [128, 128], dt_=F16)
            ident_s = load("ident", ident_d[:, :], [128, 128])
            identh_s = load("identh", identh_d[:, :], [128, 128], dt_=F16)
            dstloc_s = load("dstloc", dstloc_d[:, :], [128, W * T], dt_=F16)
            gidx_s = cp.tile([128, W * T * 8], mybir.dt.int16, tag="gidx")
            nc.sync.dma_start(gidx_s[:], gidx_d[:, :])

            ln_col = cp.tile([128, 1], F32, tag="lnc")
            nc.vector.memset(ln_col[:], LN_EPS)
            eps_col = cp.tile([128, 1], F32, tag="epsc")
            nc.vector.memset(eps_col[:], MSG_EPS)

            h_state = dp1.tile([npc, hid], F32)
            S_dram = dp1.tile([128, W * T * 128], F16)

            # ---------------- helpers ----------------
            def layer_norm(dst_ap, src_ap, g_ap, b_ap, tag, relu_eng):
                """dst <- relu(LN(src) * g + b).  src must be SBUF/PSUM f32."""
                st6 = sbm.tile([128, 6], F32, tag="lnst" + tag)
                nc.vector.bn_stats(st6[:], src_ap)
                ag2 = sbm.tile([128, 2], F32, tag="lnag" + tag)
                nc.vector.bn_aggr(ag2[:], st6[:])
                # rstd = rsqrt(var+eps): Quake seed + 2 Newton steps on DVE
                # (keeps the Scalar engine inside one ACT table set).
                a_t = sbm.tile([128, 1], F32, tag="lnva" + tag)
                nc.vector.tensor_scalar_add(a_t[:], ag2[:, 1:2], LN_EPS)
                g_t = sbm.tile([128, 1], F32, tag="lnq1" + tag)
                nc.vector.tensor_scalar(g_t[:].bitcast(I32),
                                        a_t[:].bitcast(I32), 1, None,
                                        op0=ALU.arith_shift_right)
                g2_t = sbm.tile([128, 1], F32, tag="lnq2" + tag)
                nc.vector.tensor_scalar(g2_t[:].bitcast(I32),
                                        g_t[:].bitcast(I32), -1, 0x5f3759df,
                                        op0=ALU.mult, op1=ALU.add)
                rstd = g2_t
                for it in range(2):
                    gg = sbm.tile([128, 1], F32, tag=f"lnq3{it}" + tag)
                    nc.vector.tensor_tensor(gg[:], rstd[:], rstd[:],
                                            op=ALU.mult)
                    nc.vector.tensor_tensor(gg[:], gg[:], a_t[:], op=ALU.mult)
                    nc.vector.tensor_scalar(gg[:], gg[:], -0.5, 1.5,
                                            op0=ALU.mult, op1=ALU.add)
                    gn = sbm.tile([128, 1], F32, tag=f"lnq4{it}" + tag)
                    nc.vector.tensor_tensor(gn[:], rstd[:], gg[:],
                                            op=ALU.mult)
                    rstd = gn
                # y = (x - m)*rstd on the Scalar engine (Identity is in every
                # ACT table set): y = rstd*x + (-m*rstd)
                negmr = sbm.tile([128, 1], F32, tag="lnnm" + tag)
                nc.vector.tensor_tensor(negmr[:], ag2[:, 0:1], rstd[:],
                                        op=ALU.mult)
                nc.vector.tensor_scalar_mul(negmr[:], negmr[:], -1.0)
                nfeat = src_ap.shape[-1]
                y = sbm.tile([128, nfeat], F32, tag="lny" + tag)
                nc.scalar.activation(y[:], src_ap, AF.Identity,
                                     scale=rstd[:], bias=negmr[:])
                nc.vector.tensor_tensor(y[:], y[:], g_ap, op=ALU.mult)
                nc.vector.tensor_tensor(y[:], y[:], b_ap, op=ALU.add)
                relu_eng(dst_ap, y[:])

            def srelu(dst_ap, src_ap):
                nc.scalar.activation(dst_ap, src_ap, AF.Relu)

            def vrelu(dst_ap, src_ap):
                nc.vector.tensor_scalar_max(dst_ap, src_ap, 0.0)

            def transpose128(src_ap, tag, copy_eng="scalar"):
                """PE transpose [128,128] f32 -> SBUF f16."""
                pt = pst.tile([128, 128], F32, tag="pt")
                nc.tensor.transpose(pt[:], src_ap, ident_s[:])
                st = sbm.tile([128, 128], F16, tag=tag)
                if copy_eng == "scalar":
                    nc.scalar.activation(st[:], pt[:], AF.Copy)
                else:
                    nc.vector.tensor_copy(st[:], pt[:])
                return st

            # ---------------- encode phase: h0 = x @ wnode ----------------
            def new_ag(nm):
                return dp.tile([npc, hid], F16, tag="ag", name=nm)

            def new_table(nm):
                return dp.tile([nslot, hid], F16, tag="table",
                               addr_space="Shared", name=nm)

            def emit_ag(agp, table, chunk):
                base = chunk * ag_chunk * 128
                rows = min(ag_chunk * 128, npc - base)
                tview = table[:, :].rearrange("(c n) h -> c n h", c=CORES)
                nc.gpsimd.collective_compute(
                    "AllGather", ALU.bypass,
                    ins=[agp[base:base + rows, :].opt()],
                    outs=[tview[:, base:base + rows, :].opt()],
                    replica_groups=rg)

            ag = new_ag("age")
            table = new_table("tbl0")
            for w in range(W):
                base = w * 128
                xts = sbm.tile([fin + 1, 128], F32, tag="xts")
                nc.sync.dma_start(xts[:], xT_d[:, base:base + 128])
                ph = pso.tile([128, hid], F32, tag="po")
                nc.tensor.matmul(ph[:], xts[:], wnode_s[:],
                                 start=True, stop=True)
                h0 = sbm.tile([128, hid], F16, tag="h0")
                nc.vector.tensor_copy(h0[:], ph[:])
                nc.sync.dma_start(ag[base:base + 128, :], h0[:])
                if (w + 1) % ag_chunk == 0 or w == W - 1:
                    emit_ag(ag, table, w // ag_chunk)

            # ---------------- conv layers ----------------
            for li in range(L):
                ag_next = new_ag(f"agn{li}") if li < L - 1 else None
                next_table = new_table(f"tbl{li + 1}") if li < L - 1 else None
                for w in range(W):
                    base = w * 128
                    jw = w * T

                    # gather h[src] for this window: [128, T, 128]
                    hsrc = sbw.tile([128, T, 128], F16, tag="hsrc", bufs=3)
                    for qi in range(NG):
                        c0 = qi * 8
                        ct = min(8, T - c0)
                        nc.gpsimd.dma_gather(
                            hsrc[:, c0:c0 + ct, :], table[:, :],
                            gidx_s[:, (jw + c0) * 8:(jw + c0 + ct) * 8],
                            ct * 128, ct * 128, hid,
                            queue_num=(w * NG + qi) % 4,
                            single_packet=SINGLE_PACKET)
                    attrs = sbw.tile([fe + 1, T, 128], F16, tag="attrs")
                    nc.sync.dma_start(
                        attrs[:], attrT_d[:, jw * 128:(jw + T) * 128])

                    msg = sbw.tile([128, T, 128], F16, tag="msg")
                    em = sbw.tile([128, 2, T, 128], F16, tag="em", bufs=2)
                    # quads: ea matmul + gathered-h add (identity matmul)
                    for q in range(NQ):
                        q0 = q * 4
                        qs = min(4, T - q0)
                        pq = psq.tile([128, 4, 128], F32, tag="pq")
                        for j in range(qs):
                            nc.tensor.matmul(
                                pq[:, j, :], attrs[:, q0 + j, :], wenc_s[:],
                                start=(j == 0), stop=False,
                                skip_group_check=True)
                        nc.tensor.matmul(
                            pq[:, :qs, :], identh_s[:],
                            hsrc[:, q0:q0 + qs, :],
                            start=False, stop=True, skip_group_check=True)
                        # msg = relu(ea + h_src + eps)
                        nc.scalar.activation(
                            msg[:, q0:q0 + qs, :], pq[:, :qs, :], AF.Relu,
                            bias=eps_col[:])
                    # e = exp(t * msg) ; me = msg * e
                    nc.scalar.activation(
                        em[:, 0, :, :], msg[:], AF.Exp,
                        scale=tcol_s[:, li:li + 1])
                    me_eng = nc.gpsimd if ME_ENGINE == "gpsimd" else nc.vector
                    me_eng.tensor_tensor(
                        em[:, 1, :, :], msg[:], em[:, 0, :, :], op=ALU.mult)
                    # one-hot S for the whole window (all-f16); static across
                    # layers: computed in layer 0, streamed back afterwards
                    S = sbw.tile([128, T, 128], F16, tag="S")
                    if li == 0:
                        iota_b = iota_s[:].rearrange(
                            "p (o f) -> p o f", o=1).broadcast_to([128, T, 128])
                        dl_b = dstloc_s[:, jw:jw + T].rearrange(
                            "p (t o) -> p t o", o=1).broadcast_to([128, T, 128])
                        nc.vector.tensor_tensor(S[:], iota_b, dl_b,
                                                op=ALU.is_equal)
                        nc.sync.dma_start(
                            S_dram[:, jw * 128:(jw + T) * 128],
                            S[:].rearrange("p t f -> p (t f)"))
                    else:
                        nc.sync.dma_start(
                            S[:].rearrange("p t f -> p (t f)"),
                            S_dram[:, jw * 128:(jw + T) * 128])
                    # segment accumulate [sum e | sum me] -> [128, 256] psum
                    acc = psa.tile([128, 2, hid], F32, tag="acc")
                    for g in range(T):
                        nc.tensor.matmul(
                            acc[:, :, :], S[:, g, :], em[:, :, g, :],
                            start=(g == 0), stop=(g == T - 1))

                    # agg = (sum me) / (sum e + eps)
                    sep = sbm.tile([128, hid], F32, tag="sep")
                    nc.vector.tensor_scalar_add(sep[:], acc[:, 0, :], SM_EPS)
                    rcse = sbm.tile([128, hid], F32, tag="rcse")
                    nc.vector.reciprocal_approx_fast(rcse[:], sep[:])
                    zmul = sbm.tile([128, hid], F32, tag="zmul")
                    nc.vector.tensor_tensor(zmul[:], acc[:, 1, :], rcse[:],
                                            op=ALU.mult)
                    # z += conv input rows (this core's shard of table source)
                    zin = sbm.tile([128, hid], F16, tag="zin")
                    nc.sync.dma_start(zin[:], ag[base:base + 128, :])
                    z = sbm.tile([128, hid], F32, tag="z")
                    nc.vector.tensor_tensor(z[:], zmul[:], zin[:], op=ALU.add)

                    # ---- MLP: relu(LN(z@w1+b1))@w2+b2 ----
                    zT = transpose128(z[:], "zT")
                    pu = psu.tile([128, H2], F32, tag="pu")
                    nc.tensor.matmul(pu[:], zT[:], w1_s[:, li, :],
                                     start=True, stop=True,
                                     skip_group_check=True)
                    xb = sbm.tile([128, H2], F32, tag="xb")
                    nc.vector.tensor_tensor(xb[:], pu[:],
                                            b1_s[:, li, :], op=ALU.add)
                    r = sbm.tile([128, H2], F32, tag="r")
                    layer_norm(r[:], xb[:], g1_s[:, li, :], bb1_s[:, li, :],
                               "a", srelu)
                    rT0 = transpose128(r[:, 0:128], "rT0")
                    rT1 = transpose128(r[:, 128:256], "rT1", copy_eng="vector")
                    po = pso.tile([128, hid], F32, tag="po")
                    nc.tensor.matmul(po[:], rT0[:], w2_s[:, li, 0, :],
                                     start=True, stop=False,
                                     skip_group_check=True)
                    nc.tensor.matmul(po[:], rT1[:], w2_s[:, li, 1, :],
                                     start=False, stop=True,
                                     skip_group_check=True)

                    # ---- layer epilogue: hcur = po + b2 (+ hprev) ----
                    hcur = sbm.tile([128, hid], F32, tag="hcur")
                    nc.vector.tensor_tensor(hcur[:], po[:], b2_s[:, li, :],
                                            op=ALU.add)
                    if li > 0:
                        hprev = sbm.tile([128, hid], F32, tag="hprev")
                        nc.sync.dma_start(hprev[:],
                                          h_state[base:base + 128, :])
                        nc.vector.tensor_tensor(hcur[:], hcur[:], hprev[:],
                                                op=ALU.add)
                    if li < L - 1:
                        nc.sync.dma_start(h_state[base:base + 128, :],
                                          hcur[:])
                        # z for next layer: relu(LN(h; norm[li+1]))
                        znext = sbm.tile([128, hid], F16, tag="znext")
                        layer_norm(znext[:], hcur[:], ng_s[:, li + 1, :],
                                   nb_s[:, li + 1, :], "b", srelu)
                        nc.sync.dma_start(ag_next[base:base + 128, :],
                                          znext[:])
                        if (w + 1) % ag_chunk == 0 or w == W - 1:
                            emit_ag(ag_next, next_table, w // ag_chunk)
                    else:
                        # final: relu(LN(h; norm[0])) @ lin_w + lin_b
                        fin_t = sbm.tile([128, hid], F32, tag="fin")
                        layer_norm(fin_t[:], hcur[:], ng_s[:, 0, :],
                                   nb_s[:, 0, :], "b", srelu)
                        finT = transpose128(fin_t[:], "finT")
                        pc = pso.tile([128, ncls], F32, tag="po")
                        nc.tensor.matmul(pc[:], finT[:], linw_s[:],
                                         start=True, stop=True,
                                         skip_group_check=True)
                        ow = sbm.tile([128, ncls], F32, tag="ow")
                        nc.vector.tensor_tensor(ow[:], pc[:], linb_s[:],
                                                op=ALU.add)
                        nc.sync.dma_start(out_d[base:base + 128, :], ow[:])

                if li < L - 1:
                    table = next_table
                    ag = ag_next

    nc.compile()
    return nc


# --------------------------------------------------------------------------
# entry point
# --------------------------------------------------------------------------

_CACHE = {}


def _get_program(T):
    if T not in _CACHE:
        _CACHE[T] = build_program(T)
    return _CACHE[T]


def _install_ntff_hook():
    """Bridge trn_agent_boot's ctypes NTFF profiler into antenv.axon_hooks
    (absent from this image) so run_bass_kernel_spmd(trace=True) works."""
    import types

    if "antenv.axon_hooks" in sys.modules:
        return
    try:
        sys.path.insert(0, "/root/.axon_site")
        from trn_agent_boot.trn_boot import _ntff_profile_via_ctypes

        hook = _ntff_profile_via_ctypes("/opt/axon/libaxon_pjrt.so")
    except Exception:
        hook = None
    m = types.ModuleType("antenv.axon_hooks")
    state = {"hook": hook}
    m.get_axon_ntff_profile_hook = lambda: state["hook"]
    m.set_axon_ntff_profile_hook = lambda h: state.update(hook=h)
    sys.modules["antenv.axon_hooks"] = m
    import antenv

    antenv.axon_hooks = m


def run(inputs, trace=False):
    if trace:
        _install_ntff_hook()
    T, newid, in_maps = _prep_inputs(inputs)
    nc = _get_program(T)
    res = run_bass_kernel_spmd(nc, in_maps, list(range(CORES)), trace=trace)
    full = np.concatenate([res.results[c]["out"] for c in range(CORES)],
                          axis=0)
    out = np.ascontiguousarray(full[newid])
    return out, res


def kernel(**inputs) -> np.ndarray:
    out, _ = run(inputs, trace=False)
    return out
